# revision 20
# baseline (speedup 1.0000x reference)
"""Trainium2 Bass kernel for the per-cell-MLP "MAR one-sided missingness" model.

Model (per cell (n,t) of a 1024x128 grid):
    xc     = X[n, col_idx[n,t]]
    h      = relu(W_in[n,t,:,0]*xc + W_in[n,t,:,1]*X[n,t] + b_in[n,t,:])   # [H]
    out    = sigmoid(dot(W_out[n,t,:], h) + b_out[n,t])

Sharding: rows N split across 8 cores (NR=128 rows each), fully data parallel.

HBM-bound problem (4 weight tensors of [N,T,H] stream once). All four are
host-packed to fp16 into ONE DRAM tensor WPACK[t, 4*H*NR] so each
superblock of G rows arrives as a single DMA (2 MB at G=16); fp16 weights
keep the end-to-end rel err ~9e-3 (< 2e-2 gate). xc is staged host-side
(pure index re-encoding of col_idx, replacing the baseline's one-hot
masks and their 2 MB/core of traffic).

Per-core layout: partitions = t (128). w0/w1/b use free = (h, g) with g
innermost so the broadcast multiplies run in the DVE's packed 16-bit mode
(2 elem/cycle); wo and the drain use (g, h) so the h-reduction is an
innermost-axis tensor_reduce.

Per superblock (software-pipelined; back(s-1) emitted before front(s)):
  DMA  : WPACK column slice -> wblk                 (one transfer)
  DVE  : m1 = w1 * bcast(x),  a0 = w0 * bcast(xc)   (fp16 packed TT)
  PE   : psum_c = I@m1_c + I@a0_c + I@b_c           (fp32 accumulate,
         [128,512] chunks = 1 PSUM bank each)
  ACT  : urelu[(g,h)] = relu(psum) via one transposed-read drain -> fp16
  Pool : r = urelu * wo                             (fp16)
  DVE  : red[:, block] = reduce_h(r)                (fp32 out)
Epilogue: out = sigmoid(red + b_out^T), DMA out, host transposes back.
Block 0's DMA is split per-segment so w1/w0 land early (shorter fill);
the last superblock is split in half to shorten the serial tail.

Streams ~16.3 MB per core -> ~46 us DMA floor at 358 GB/s.
"""

import numpy as np

N, T, H = 1024, 128, 128
M = 8            # cores
NR = N // M      # rows per core
G = 16           # rows per full superblock
HG = H * G

# (g0, glen) schedule: full blocks, then two half blocks to shorten the tail
BLOCKS = [(g0, G) for g0 in range(0, NR - G, G)] + [(NR - G, G // 2), (NR - G // 2, G // 2)]

_cache = {}


def _build():
    if "nc" in _cache:
        return _cache["nc"]
    import concourse.bacc as bacc
    import concourse.mybir as mybir
    import concourse.tile as tile

    f32 = mybir.dt.float32
    f16 = mybir.dt.float16
    Alu = mybir.AluOpType
    Act = mybir.ActivationFunctionType

    nc = bacc.Bacc()
    # column layout per block: [4, H, glen] fp16 (w0, w1, b in (h,g); wo in (g,h))
    wpack = nc.declare_dram_parameter("wpack", [T, 4 * H * NR], f16, isOutput=False)
    xcx16 = nc.declare_dram_parameter("xcx16", [T, 2 * NR], f16, isOutput=False)
    bout = nc.declare_dram_parameter("bout", [T, NR], f32, isOutput=False)
    ident = nc.declare_dram_parameter("ident", [128, 128], f16, isOutput=False)
    out = nc.declare_dram_parameter("out", [T, NR], f32, isOutput=True)

    with tile.TileContext(nc) as tc:
        with (
            tc.tile_pool(name="const", bufs=1) as constp,
            tc.tile_pool(name="wp", bufs=4) as wpool,
            tc.tile_pool(name="comp", bufs=2) as comp,
            tc.tile_pool(name="up", bufs=4) as upool,
            tc.tile_pool(name="rp", bufs=3) as rpool,
            tc.tile_pool(name="acc", bufs=1) as accp,
            tc.tile_pool(name="ps", bufs=2, space="PSUM") as pspool,
        ):
            # Consts ride the same HWDGE queue as the weight stream, ordered
            # by first use: xcx feeds the first multiply; ident is only
            # needed by the first matmul so it queues after block 0's w0/w1;
            # bout is only read by the epilogue and queues dead last.
            xcx_sb = constp.tile([T, 2 * NR], f16)
            nc.sync.dma_start(xcx_sb[:], xcx16[:])
            id_sb = constp.tile([128, 128], f16)
            bo_sb = constp.tile([T, NR], f32)
            sigw = constp.tile([128, 1], f32)

            red = accp.tile([T, NR], f32)

            def front(bi):
                g0, gl = BLOCKS[bi]
                hg = H * gl
                col0 = 4 * H * g0
                wblk = wpool.tile([128, 4 * HG], f16, tag="w")
                if bi == 0:
                    # segment-split first transfer: w0/w1 land first for the
                    # multiply; ident slots in before b (needed by matmul 1).
                    for seg in (0, 1):
                        nc.sync.dma_start(
                            wblk[:, seg * hg : (seg + 1) * hg],
                            wpack[:, col0 + seg * hg : col0 + (seg + 1) * hg],
                        )
                    nc.sync.dma_start(id_sb[:], ident[:])
                    # warm the sigmoid activation table long before the
                    # epilogue needs it (the lazy load costs 1.3 us).
                    nc.scalar.activation(sigw[:], xcx_sb[:, 0:1], Act.Sigmoid)
                    for seg in (2, 3):
                        nc.sync.dma_start(
                            wblk[:, seg * hg : (seg + 1) * hg],
                            wpack[:, col0 + seg * hg : col0 + (seg + 1) * hg],
                        )
                else:
                    nc.sync.dma_start(
                        wblk[:, 0 : 4 * hg], wpack[:, col0 : col0 + 4 * hg]
                    )
                if bi == len(BLOCKS) - 1:
                    nc.sync.dma_start(bo_sb[:], bout[:])
                bfl = wblk[:, 2 * hg : 3 * hg]
                wov = wblk[:, 3 * hg : 4 * hg]  # flat (g, h) order

                nsl = slice(g0, g0 + gl)
                # one packed TT computes both products: j=0 -> w0*xc, j=1 -> w1*x
                xj = (
                    xcx_sb[:]
                    .rearrange("p (j n) -> p j n", j=2)[:, :, nsl]
                    .broadcast_to([128, 2, gl, H])
                    .rearrange("p j g h -> p j h g")
                )
                wj = wblk[:, 0 : 2 * hg].rearrange(
                    "p (j h g) -> p j h g", j=2, g=gl
                )
                ma = comp.tile([128, 2 * HG], f16, tag="ma")
                mav = ma[:, 0 : 2 * hg].rearrange("p (j h g) -> p j h g", j=2, g=gl)
                nc.vector.tensor_tensor(mav, wj, xj, Alu.mult)

                # per-chunk PSUM tiles (1 bank each) keep the PE<->ACT
                # rotation fine-grained; drains read the chunk g-major so
                # the fp16 write lands unit-stride in the (g, h) urelu tile.
                urelu = upool.tile([128, HG], f16, tag="u")
                ugh = urelu[:, 0:hg].rearrange("p (g h) -> p g h", g=gl)
                hc = 512 // gl  # h-rows per 512-col PSUM chunk
                for c in range(hg // 512):
                    csl = slice(c * 512, (c + 1) * 512)
                    ps = pspool.tile([128, 512], f32, tag=f"ps{c % 4}")
                    nc.tensor.matmul(ps[:], id_sb[:], ma[:, csl], start=True, stop=False)
                    nc.tensor.matmul(
                        ps[:], id_sb[:], ma[:, hg + c * 512 : hg + (c + 1) * 512],
                        start=False, stop=False,
                    )
                    nc.tensor.matmul(ps[:], id_sb[:], bfl[:, csl], start=False, stop=True)
                    psv = ps[:].rearrange("p (h g) -> p g h", g=gl)
                    nc.scalar.activation(
                        ugh[:, :, hc * c : hc * (c + 1)], psv, Act.Relu
                    )
                return (bi, urelu, wov)

            def back(st):
                bi, urelu, wov = st
                g0, gl = BLOCKS[bi]
                hg = H * gl
                r = rpool.tile([128, HG], f16, tag="rg")
                nc.vector.tensor_tensor(r[:, 0:hg], urelu[:, 0:hg], wov, Alu.mult)
                nc.vector.tensor_reduce(
                    red[:, g0 : g0 + gl],
                    r[:, 0:hg].rearrange("p (g h) -> p g h", g=gl),
                    axis=mybir.AxisListType.X,
                    op=Alu.add,
                )

            # two-deep software pipeline, front-first: each cycle emits
            # front(s) (DMA, multiply, PE, drain) and then back(s-2)
            # (r, reduce). m1a0(s) precedes back-work in the DVE queue so
            # the PE is fed without waiting on the previous blocks' reduce,
            # and back(s-2)'s inputs are two blocks old - always ready.
            states = []
            for bi in range(len(BLOCKS)):
                states.append(front(bi))
                if len(states) > 2:
                    back(states.pop(0))
            for st in states:
                back(st)

            lg = comp.tile([T, NR], f32, tag="lg")
            nc.vector.tensor_tensor(lg[:], red[:], bo_sb[:], Alu.add)
            ot = comp.tile([T, NR], f32, tag="ot")
            nc.scalar.activation(ot[:], lg[:], Act.Sigmoid)
            nc.sync.dma_start(out[:], ot[:])

    nc.compile()
    _cache["nc"] = nc
    return nc


def make_in_maps(X, W_in, b_in, W_out, b_out, col_idx):
    f16 = np.float16
    X = np.asarray(X, dtype=np.float32)
    b_out = np.asarray(b_out, dtype=np.float32)
    col_idx = np.asarray(col_idx)
    xc = np.take_along_axis(X, col_idx, axis=1)

    w0_16 = np.asarray(W_in)[:, :, :, 0].astype(f16)   # [N, T, H]
    w1_16 = np.asarray(W_in)[:, :, :, 1].astype(f16)
    b_16 = np.asarray(b_in).astype(f16)
    wo_16 = np.asarray(W_out).astype(f16)
    ident = np.eye(128, dtype=f16)

    in_maps = []
    for c in range(M):
        sl = slice(c * NR, (c + 1) * NR)
        # per block: [T, 4, H, glen] (w0,w1,b in (h,g); wo in (g,h)), blocks
        # concatenated along columns
        w0c = w0_16[sl].transpose(1, 2, 0)   # [T, H, NR]
        w1c = w1_16[sl].transpose(1, 2, 0)
        bc = b_16[sl].transpose(1, 2, 0)
        woc = wo_16[sl].transpose(1, 0, 2)   # [T, NR, H]
        cols = []
        for g0, gl in BLOCKS:
            gs = slice(g0, g0 + gl)
            cols.append(
                np.concatenate(
                    [
                        w0c[:, :, gs].reshape(T, -1),
                        w1c[:, :, gs].reshape(T, -1),
                        bc[:, :, gs].reshape(T, -1),
                        woc[:, gs, :].reshape(T, -1),
                    ],
                    axis=1,
                )
            )
        wpack = np.ascontiguousarray(np.concatenate(cols, axis=1))
        assert wpack.shape == (T, 4 * H * NR)
        xcx = np.concatenate(
            [xc[sl].T.astype(f16), X[sl].T.astype(f16)], axis=1
        )  # j=0 -> xc (pairs w0), j=1 -> x (pairs w1)
        in_maps.append(
            {
                "wpack": wpack,
                "xcx16": np.ascontiguousarray(xcx),
                "bout": np.ascontiguousarray(b_out[sl].T),
                "ident": ident,
            }
        )
    return in_maps


def kernel(X, W_in, b_in, W_out, b_out, col_idx):
    from concourse.bass_utils import run_bass_kernel_spmd

    nc = _build()
    in_maps = make_in_maps(X, W_in, b_in, W_out, b_out, col_idx)
    res = run_bass_kernel_spmd(nc, in_maps, list(range(M))).results
    out = np.empty((N, T), np.float32)
    for c in range(M):
        out[c * NR : (c + 1) * NR] = res[c]["out"].T
    return out


# revision 23
# speedup vs baseline: 1.0549x; 1.0549x over previous
"""Trainium2 Bass kernel for the per-cell-MLP "MAR one-sided missingness" model.

Model (per cell (n,t) of a 1024x128 grid):
    xc     = X[n, col_idx[n,t]]
    h      = relu(W_in[n,t,:,0]*xc + W_in[n,t,:,1]*X[n,t] + b_in[n,t,:])   # [H]
    out    = sigmoid(dot(W_out[n,t,:], h) + b_out[n,t])

Sharding: rows N split across 8 cores (NR=128 rows each), fully data parallel.

HBM-bound problem (4 weight tensors of [N,T,H] stream once). All four are
host-packed to fp16 into ONE DRAM tensor WPACK[t, 4*H*NR] so each
superblock of G rows arrives as a single DMA (2 MB at G=16); fp16 weights
keep the end-to-end rel err ~9e-3 (< 2e-2 gate). xc is staged host-side
(pure index re-encoding of col_idx, replacing the baseline's one-hot
masks and their 2 MB/core of traffic).

Per-core layout: partitions = t (128). w0/w1/b use free = (h, g) with g
innermost so the broadcast multiplies run in the DVE's packed 16-bit mode
(2 elem/cycle); wo and the drain use (g, h) so the h-reduction is an
innermost-axis tensor_reduce.

Per superblock (software-pipelined; back(s-1) emitted before front(s)):
  DMA  : WPACK column slice -> wblk                 (one transfer)
  DVE  : m1 = w1 * bcast(x),  a0 = w0 * bcast(xc)   (fp16 packed TT)
  PE   : psum_c = I@m1_c + I@a0_c + I@b_c           (fp32 accumulate,
         [128,512] chunks = 1 PSUM bank each)
  ACT  : urelu[(g,h)] = relu(psum) via one transposed-read drain -> fp16
  Pool : r = urelu * wo                             (fp16)
  DVE  : red[:, block] = reduce_h(r)                (fp32 out)
Epilogue: out = sigmoid(red + b_out^T), DMA out, host transposes back.
Block 0's DMA is split per-segment so w1/w0 land early (shorter fill);
the last superblock is split in half to shorten the serial tail.

Streams ~16.3 MB per core -> ~46 us DMA floor at 358 GB/s.
"""

import numpy as np

N, T, H = 1024, 128, 128
M = 8            # cores
NR = N // M      # rows per core
G = 16           # rows per full superblock
HG = H * G

# (g0, glen) schedule: full blocks, then two half blocks to shorten the tail
BLOCKS = [(g0, G) for g0 in range(0, NR - G, G)] + [(NR - G, G // 2), (NR - G // 2, G // 2)]

_cache = {}


def _build():
    if "nc" in _cache:
        return _cache["nc"]
    import concourse.bacc as bacc
    import concourse.mybir as mybir
    import concourse.tile as tile

    f32 = mybir.dt.float32
    f16 = mybir.dt.float16
    Alu = mybir.AluOpType
    Act = mybir.ActivationFunctionType

    nc = bacc.Bacc()
    # column layout per block: [4, H, glen] fp16 (w0, w1, b in (h,g); wo in (g,h))
    wpack = nc.declare_dram_parameter("wpack", [T, 4 * H * NR], f16, isOutput=False)
    xcx16 = nc.declare_dram_parameter("xcx16", [T, 2 * NR], f16, isOutput=False)
    bout = nc.declare_dram_parameter("bout", [T, NR], f32, isOutput=False)
    ident = nc.declare_dram_parameter("ident", [128, 128], f16, isOutput=False)
    out = nc.declare_dram_parameter("out", [T, NR], f32, isOutput=True)

    with tile.TileContext(nc) as tc:
        with (
            tc.tile_pool(name="const", bufs=1) as constp,
            tc.tile_pool(name="wp", bufs=5) as wpool,
            tc.tile_pool(name="comp", bufs=2) as comp,
            tc.tile_pool(name="up", bufs=4) as upool,
            tc.tile_pool(name="rp", bufs=3) as rpool,
            tc.tile_pool(name="acc", bufs=1) as accp,
            tc.tile_pool(name="ps", bufs=2, space="PSUM") as pspool,
        ):
            # Consts ride the same HWDGE queue as the weight stream, ordered
            # by first use: xcx feeds the first multiply; ident is only
            # needed by the first matmul so it queues after block 0's w0/w1;
            # bout is only read by the epilogue and queues dead last.
            xcx_sb = constp.tile([T, 2 * NR], f16)
            nc.sync.dma_start(xcx_sb[:], xcx16[:])
            id_sb = constp.tile([128, 128], f16)
            bo_sb = constp.tile([T, NR], f32)
            sigw = constp.tile([128, 1], f32)

            red = accp.tile([T, NR], f32)

            def front(bi):
                g0, gl = BLOCKS[bi]
                hg = H * gl
                col0 = 4 * H * g0
                wblk = wpool.tile([128, 4 * HG], f16, tag="w")
                if bi == 0:
                    # segment-split first transfer: w0/w1 land first for the
                    # multiply; ident slots in before b (needed by matmul 1).
                    for seg in (0, 1):
                        nc.sync.dma_start(
                            wblk[:, seg * hg : (seg + 1) * hg],
                            wpack[:, col0 + seg * hg : col0 + (seg + 1) * hg],
                        )
                    nc.sync.dma_start(id_sb[:], ident[:])
                    # warm the sigmoid activation table long before the
                    # epilogue needs it (the lazy load costs 1.3 us).
                    nc.scalar.activation(sigw[:], xcx_sb[:, 0:1], Act.Sigmoid)
                    for seg in (2, 3):
                        nc.sync.dma_start(
                            wblk[:, seg * hg : (seg + 1) * hg],
                            wpack[:, col0 + seg * hg : col0 + (seg + 1) * hg],
                        )
                else:
                    nc.sync.dma_start(
                        wblk[:, 0 : 4 * hg], wpack[:, col0 : col0 + 4 * hg]
                    )
                if bi == len(BLOCKS) - 1:
                    nc.sync.dma_start(bo_sb[:], bout[:])
                bfl = wblk[:, 2 * hg : 3 * hg]
                wov = wblk[:, 3 * hg : 4 * hg]  # flat (g, h) order

                nsl = slice(g0, g0 + gl)
                # one packed TT computes both products: j=0 -> w0*xc, j=1 -> w1*x
                xj = (
                    xcx_sb[:]
                    .rearrange("p (j n) -> p j n", j=2)[:, :, nsl]
                    .broadcast_to([128, 2, gl, H])
                    .rearrange("p j g h -> p j h g")
                )
                wj = wblk[:, 0 : 2 * hg].rearrange(
                    "p (j h g) -> p j h g", j=2, g=gl
                )
                ma = comp.tile([128, 2 * HG], f16, tag="ma")
                mav = ma[:, 0 : 2 * hg].rearrange("p (j h g) -> p j h g", j=2, g=gl)
                nc.vector.tensor_tensor(mav, wj, xj, Alu.mult)

                # per-chunk PSUM tiles (1 bank each) keep the PE<->ACT
                # rotation fine-grained; drains read the chunk g-major so
                # the fp16 write lands unit-stride in the (g, h) urelu tile.
                urelu = upool.tile([128, HG], f16, tag="u")
                ugh = urelu[:, 0:hg].rearrange("p (g h) -> p g h", g=gl)
                hc = 512 // gl  # h-rows per 512-col PSUM chunk
                for c in range(hg // 512):
                    csl = slice(c * 512, (c + 1) * 512)
                    ps = pspool.tile([128, 512], f32, tag=f"ps{c % 4}")
                    nc.tensor.matmul(ps[:], id_sb[:], ma[:, csl], start=True, stop=False)
                    nc.tensor.matmul(
                        ps[:], id_sb[:], ma[:, hg + c * 512 : hg + (c + 1) * 512],
                        start=False, stop=False,
                    )
                    nc.tensor.matmul(ps[:], id_sb[:], bfl[:, csl], start=False, stop=True)
                    psv = ps[:].rearrange("p (h g) -> p g h", g=gl)
                    nc.scalar.activation(
                        ugh[:, :, hc * c : hc * (c + 1)], psv, Act.Relu
                    )
                return (bi, urelu, wov)

            def back(st):
                bi, urelu, wov = st
                g0, gl = BLOCKS[bi]
                hg = H * gl
                r = rpool.tile([128, HG], f16, tag="rg")
                nc.vector.tensor_tensor(r[:, 0:hg], urelu[:, 0:hg], wov, Alu.mult)
                # hybrid reduce: two packed fp16 halving adds, then a small
                # fp32 tensor_reduce (tensor_reduce only has a 1x uop, so
                # shrinking its input 4x is cheaper than reducing directly)
                rv = r[:, 0:hg].rearrange("p (g h) -> p g h", g=gl)
                t1 = rpool.tile([128, HG // 2], f16, tag="t1")
                t1v = t1[:, 0 : hg // 2].rearrange("p (g h) -> p g h", g=gl)
                nc.vector.tensor_tensor(t1v, rv[:, :, 0:64], rv[:, :, 64:128], Alu.add)
                t2 = rpool.tile([128, HG // 4], f16, tag="t2")
                t2v = t2[:, 0 : hg // 4].rearrange("p (g h) -> p g h", g=gl)
                nc.vector.tensor_tensor(t2v, t1v[:, :, 0:32], t1v[:, :, 32:64], Alu.add)
                nc.vector.tensor_reduce(
                    red[:, g0 : g0 + gl],
                    t2v,
                    axis=mybir.AxisListType.X,
                    op=Alu.add,
                )

            # two-deep software pipeline, front-first: each cycle emits
            # front(s) (DMA, multiply, PE, drain) and then back(s-2)
            # (r, reduce). m1a0(s) precedes back-work in the DVE queue so
            # the PE is fed without waiting on the previous blocks' reduce,
            # and back(s-2)'s inputs are two blocks old - always ready.
            states = []
            for bi in range(len(BLOCKS)):
                if len(states) >= 2:
                    back(states.pop(0))
                states.append(front(bi))
            for st in states:
                back(st)

            lg = comp.tile([T, NR], f32, tag="lg")
            nc.vector.tensor_tensor(lg[:], red[:], bo_sb[:], Alu.add)
            ot = comp.tile([T, NR], f32, tag="ot")
            nc.scalar.activation(ot[:], lg[:], Act.Sigmoid)
            nc.sync.dma_start(out[:], ot[:])

    nc.compile()
    _cache["nc"] = nc
    return nc


def make_in_maps(X, W_in, b_in, W_out, b_out, col_idx):
    f16 = np.float16
    X = np.asarray(X, dtype=np.float32)
    b_out = np.asarray(b_out, dtype=np.float32)
    col_idx = np.asarray(col_idx)
    xc = np.take_along_axis(X, col_idx, axis=1)

    w0_16 = np.asarray(W_in)[:, :, :, 0].astype(f16)   # [N, T, H]
    w1_16 = np.asarray(W_in)[:, :, :, 1].astype(f16)
    b_16 = np.asarray(b_in).astype(f16)
    wo_16 = np.asarray(W_out).astype(f16)
    ident = np.eye(128, dtype=f16)

    in_maps = []
    for c in range(M):
        sl = slice(c * NR, (c + 1) * NR)
        # per block: [T, 4, H, glen] (w0,w1,b in (h,g); wo in (g,h)), blocks
        # concatenated along columns
        w0c = w0_16[sl].transpose(1, 2, 0)   # [T, H, NR]
        w1c = w1_16[sl].transpose(1, 2, 0)
        bc = b_16[sl].transpose(1, 2, 0)
        woc = wo_16[sl].transpose(1, 0, 2)   # [T, NR, H]
        cols = []
        for g0, gl in BLOCKS:
            gs = slice(g0, g0 + gl)
            cols.append(
                np.concatenate(
                    [
                        w0c[:, :, gs].reshape(T, -1),
                        w1c[:, :, gs].reshape(T, -1),
                        bc[:, :, gs].reshape(T, -1),
                        woc[:, gs, :].reshape(T, -1),
                    ],
                    axis=1,
                )
            )
        wpack = np.ascontiguousarray(np.concatenate(cols, axis=1))
        assert wpack.shape == (T, 4 * H * NR)
        xcx = np.concatenate(
            [xc[sl].T.astype(f16), X[sl].T.astype(f16)], axis=1
        )  # j=0 -> xc (pairs w0), j=1 -> x (pairs w1)
        in_maps.append(
            {
                "wpack": wpack,
                "xcx16": np.ascontiguousarray(xcx),
                "bout": np.ascontiguousarray(b_out[sl].T),
                "ident": ident,
            }
        )
    return in_maps


def kernel(X, W_in, b_in, W_out, b_out, col_idx):
    from concourse.bass_utils import run_bass_kernel_spmd

    nc = _build()
    in_maps = make_in_maps(X, W_in, b_in, W_out, b_out, col_idx)
    res = run_bass_kernel_spmd(nc, in_maps, list(range(M))).results
    out = np.empty((N, T), np.float32)
    for c in range(M):
        out[c * NR : (c + 1) * NR] = res[c]["out"].T
    return out
